# revision 18
# baseline (speedup 1.0000x reference)
"""Trainium2 Bass kernel for BasicTransformerBlock_Epipolar (relative-position attention).

Math (per batch b, head h):
  q = x@Wq, k = x@Wk, v = x@Wv            (H=16 heads, dh=64)
  sim[t,s]  = (q[t]·k[s] + q[t]·Tk[s-t+1024]) * dh^-0.5
  attn      = softmax_s(sim)
  out[t]    = sum_s attn[t,s]*v[s] + sum_s attn[t,s]*Tv[s-t+1024]
  y = out@Wo + bo

Sharding: data-parallel over batch, 2 batches per core, 8 cores, no collectives.

v2 notes (vs the first working version):
  - The PE's HAM clock gate ignores transpose-mode activity, so an attention
    phase whose real matmuls come in sparse bursts runs at K=4/8 (1.2 GHz).
    Projection matmul groups are interleaved into the attention stream
    ("feeder") so every HAM window sees real matmul activity: b0-attention
    carries part of b1's q/k/v projections, b1-attention carries the rest
    plus b0's output projection; b1's output projection forms a dense tail.
  - The rel-k skew addition (sim += msk) is done on the PE as an
    identity-matmul accumulation into the sim PSUM instead of a DVE
    tensor_tensor, and exp() reads the PSUM directly.
  - Bounce buffers: rbuf in fp8e4 (R values are O(1)), abuf in bf16.
    qT/kT/aot/weights in bf16.

The relative-position terms need a "skew" (diagonal remap) which SBUF access
patterns cannot express; both are routed through DRAM with strided access
patterns:
  - R[t,r] = q[t]·Tk[r] is computed blockwise as a plain matmul, written to a
    row-stride-2049 buffer (fp8), and read back with row stride 2048, which
    yields exactly M[t,s] = R[t, s-t+1024].
  - e = exp(SCALE*sim) (unnormalized attn) is written contiguously (row
    stride 1024, bf16) and read back with row stride 1025, which yields
    A_skew[i,j] = e[i, i+j-127]; PE-transposed chunks of it contract with Tv
    into the same PSUM as attn@v.  Softmax normalization is deferred to the
    PSUM eviction (one multiply by a broadcast 1/l row).
"""

import sys

sys.path.insert(0, "/opt/trn_rl_repo")

from itertools import chain

import numpy as np

import concourse.bass as bass
import concourse.tile as tile
from concourse import bacc, mybir
from concourse.bass_utils import run_bass_kernel_spmd
from concourse.masks import make_identity

FP = mybir.dt.float32
FR = mybir.dt.float32r
BF = mybir.dt.bfloat16
F8 = mybir.dt.float8e4

B, T, D = 16, 1024, 1024
H, DH = 16, 64
NCORE = 8
BL = B // NCORE          # batches per core
TL = BL * T              # local token rows
SCALE = DH ** -0.5
NHP = H // 2             # head pairs
NBLK = T // 128          # 128-row blocks per batch
WREL = 1151              # rel window width per 128 t-block
RSTRIDE = 2049
RSEG = T * RSTRIDE       # rbuf elements per (b, h)
AGUARD = 128
ASEG = AGUARD + 128 * 1024 + AGUARD  # abuf elements per (b, h, blk)


def _ap(t_ap, offset, pattern):
    return bass.AP(tensor=t_ap.tensor, offset=offset, ap=pattern)


def build(num_b=BL, num_hp=NHP, num_blk=NBLK):
    nc = bacc.Bacc("TRN2", target_bir_lowering=False, debug=False, num_devices=NCORE)

    x = nc.dram_tensor("x", [BL, T, D], FP, kind="ExternalInput").ap()
    wq = nc.dram_tensor("Wq", [D, D], FP, kind="ExternalInput").ap()
    wk = nc.dram_tensor("Wk", [D, D], FP, kind="ExternalInput").ap()
    wv = nc.dram_tensor("Wv", [D, D], FP, kind="ExternalInput").ap()
    wo = nc.dram_tensor("Wo", [D, D], FP, kind="ExternalInput").ap()
    bo = nc.dram_tensor("bo", [D], FP, kind="ExternalInput").ap()
    tk = nc.dram_tensor("rel_k_table", [2 * T + 1, DH], FP, kind="ExternalInput").ap()
    tv = nc.dram_tensor("rel_v_table", [2 * T + 1, DH], FP, kind="ExternalInput").ap()
    y = nc.dram_tensor("y", [BL, T, D], FP, kind="ExternalOutput").ap()

    qT = nc.dram_tensor("qT", [D, TL], BF).ap()
    kT = nc.dram_tensor("kT", [D, TL], BF).ap()
    vB = nc.dram_tensor("vB", [TL, D], BF).ap()
    aot = nc.dram_tensor("aot", [D, TL], BF).ap()
    rbuf = nc.dram_tensor("rbuf", [BL * H * RSEG], F8).ap()
    abuf = nc.dram_tensor("abuf", [BL * H * NBLK * ASEG], BF).ap()

    with tile.TileContext(nc) as tc:
        const = tc.alloc_tile_pool(name="const", bufs=1)
        ps512 = tc.alloc_tile_pool(name="ps512", bufs=6, space="PSUM")
        pso = tc.alloc_tile_pool(name="pso", bufs=2, space="PSUM")

        # ---- constants ----
        ident = const.tile([128, 128], FP, tag="ident")
        make_identity(nc, ident)
        ident_bf = const.tile([128, 128], BF, tag="ident_bf")
        nc.vector.tensor_copy(ident_bf, ident)
        ident_f8 = const.tile([128, 128], F8, tag="ident_f8")
        nc.vector.tensor_copy(ident_f8, ident)

        ones_i8 = const.tile([128, 128], mybir.dt.int8, tag="ones_i8")
        nc.vector.memset(ones_i8, 1)
        zeros_bf = const.tile([128, 128], BF, tag="zeros_bf")
        nc.vector.memset(zeros_bf, 0.0)
        # mask_lo[p,f] = 1 if p+f >= 127 ; mask_hi[p,f] = 1 if p+f <= 126
        # invalid corners of the skewed-attn read: chunk 0 is invalid where
        # p+f <= 126 (use mask_hi to zero), chunk 8 invalid where p+f >= 127.
        mask_lo = const.tile([128, 128], mybir.dt.int8, tag="mask_lo")
        nc.gpsimd.affine_select(
            out=mask_lo, in_=ones_i8, pattern=[[1, 128]],
            compare_op=mybir.AluOpType.is_ge, fill=0, base=-127,
            channel_multiplier=1,
        )
        mask_hi = const.tile([128, 128], mybir.dt.int8, tag="mask_hi")
        nc.gpsimd.affine_select(
            out=mask_hi, in_=ones_i8, pattern=[[-1, 128]],
            compare_op=mybir.AluOpType.is_ge, fill=0, base=126,
            channel_multiplier=-1,
        )

        # bo broadcast to all partitions
        bo128 = const.tile([128, D], FP, tag="bo128")
        nc.sync.dma_start(out=bo128, in_=_ap(bo, 0, [[0, 128], [1, D]]))

        # ================= weights + x^T residency =================
        pw = tc.alloc_tile_pool(name="pw", bufs=1)
        pev = tc.alloc_tile_pool(name="pev", bufs=4)
        pa = tc.alloc_tile_pool(name="pa", bufs=2)

        # Tk^T resident in SBUF (bf16), duplicated across both partition
        # halves so it can feed row-packed matmuls for either head.
        tk_tmp = pa.tile([128, 16, DH], FP, tag="tk_tmp")
        nc.sync.dma_start(
            out=tk_tmp, in_=tk[0 : 16 * 128, :].rearrange("(c p) d -> p c d", p=128)
        )
        tkT = const.tile([128, 16 * 128 + 4], BF, tag="tkT")
        for c in range(16):
            ptile = ps512.tile([128, 512], FP, tag="ps512")
            nc.tensor.transpose(ptile[0:DH, 0:128], tk_tmp[:, c, :], ident)
            nc.scalar.copy(out=tkT[0:DH, c * 128 : (c + 1) * 128], in_=ptile[0:DH, 0:128])
            nc.scalar.copy(out=tkT[DH:128, c * 128 : (c + 1) * 128], in_=ptile[0:DH, 0:128])

        nc.scalar.copy(out=tkT[:, 16 * 128 : 16 * 128 + 4], in_=zeros_bf[:, 0:4])

        # Tv rows 1..2048 as bf16 chunks: tv_bf[p, m, d] = tv[1 + 128m + p, d]
        tv_tmp = pa.tile([128, 16, DH], FP, tag="tv_tmp")
        nc.sync.dma_start(
            out=tv_tmp, in_=tv[1 : 1 + 16 * 128, :].rearrange("(m p) d -> p m d", p=128)
        )
        tv_bf = const.tile([128, 16, DH], BF, tag="tv_bf")
        nc.scalar.copy(out=tv_bf, in_=tv_tmp)

        # x^T resident (bf16): xT[p, ic, tg] = x[tg//T, tg%T, 128*ic + p]
        xT = pw.tile([128, 8, TL], BF, tag="xT")
        for b in range(num_b):
            for tb in range(8):
                xt = pa.tile([128, D], FP, tag="xt")
                nc.sync.dma_start(out=xt, in_=x[b, tb * 128 : (tb + 1) * 128, :])
                for ic in range(8):
                    ptile = ps512.tile([128, 512], FP, tag="ps512")
                    nc.tensor.transpose(ptile[:, 0:128], xt[:, ic * 128 : (ic + 1) * 128], ident)
                    nc.scalar.copy(
                        out=xT[:, ic, (b * 8 + tb) * 128 : (b * 8 + tb + 1) * 128],
                        in_=ptile[:, 0:128],
                    )

        # weights resident in bf16, cast f32->bf16 in the DMA engine (SWDGE);
        # the wv slot is later reused for wo.
        def load_w(w_ap, tag):
            wsb = pw.tile([128, 8, D], BF, tag=tag)
            nc.gpsimd.dma_start(out=wsb, in_=w_ap.rearrange("(c p) j -> p c j", p=128))
            return wsb

        wsb_q = load_w(wq, "wsb_q")
        wsb_k = load_w(wk, "wsb_k")
        wsb_v = load_w(wv, "wsb_vo")

        # ---- projection group generators (each yield ≈ 2 matmuls) ----
        def qk_group(wsb, dst, jt, tt):
            ps = ps512.tile([128, 512], FP, tag="ps512")
            for ic2 in range(4):
                for ic in (2 * ic2, 2 * ic2 + 1):
                    nc.tensor.matmul(
                        ps,
                        lhsT=wsb[:, ic, jt * 128 : (jt + 1) * 128],
                        rhs=xT[:, ic, tt * 512 : (tt + 1) * 512],
                        start=(ic == 0),
                        stop=(ic == 7),
                    )
                yield
            ev = pev.tile([128, 512], BF, tag="ev")
            nc.scalar.copy(out=ev, in_=ps)
            nc.sync.dma_start(
                out=dst[jt * 128 : (jt + 1) * 128, tt * 512 : (tt + 1) * 512],
                in_=ev,
            )
            yield

        def v_group(wsb, tt, jh):
            ps = ps512.tile([128, 512], FP, tag="ps512")
            for ic2 in range(4):
                for ic in (2 * ic2, 2 * ic2 + 1):
                    nc.tensor.matmul(
                        ps,
                        lhsT=xT[:, ic, tt * 128 : (tt + 1) * 128],
                        rhs=wsb[:, ic, jh * 512 : (jh + 1) * 512],
                        start=(ic == 0),
                        stop=(ic == 7),
                    )
                yield
            ev = pev.tile([128, 512], BF, tag="evb")
            nc.scalar.copy(out=ev, in_=ps)
            nc.sync.dma_start(
                out=vB[tt * 128 : (tt + 1) * 128, jh * 512 : (jh + 1) * 512],
                in_=ev,
            )
            yield

        def oproj_block(wsb, b, k):
            tt = b * 8 + k
            asb = pev.tile([128, 8, 128], BF, tag="asb")
            nc.sync.dma_start(
                out=asb,
                in_=aot[:, tt * 128 : (tt + 1) * 128].rearrange(
                    "(c p) t -> p c t", p=128
                ),
            )
            yield
            for eh in range(2):
                ps = ps512.tile([128, 512], FP, tag="ps512")
                for ic2 in range(4):
                    for ic in (2 * ic2, 2 * ic2 + 1):
                        nc.tensor.matmul(
                            ps,
                            lhsT=asb[:, ic, :],
                            rhs=wsb[:, ic, eh * 512 : (eh + 1) * 512],
                            start=(ic == 0),
                            stop=(ic == 7),
                        )
                    yield
                ysb = pev.tile([128, 512], FP, tag="ysb")
                nc.vector.tensor_add(ysb, ps, bo128[:, eh * 512 : (eh + 1) * 512])
                nc.sync.dma_start(
                    out=y[b, k * 128 : (k + 1) * 128, eh * 512 : (eh + 1) * 512],
                    in_=ysb,
                )
                yield

        # ---- phase A: b0's projections, emitted densely ----
        def run_all(gens):
            for g in gens:
                for _ in g:
                    pass

        b0_gens = []
        for jt in range(8):
            for tt in (0, 1):
                b0_gens.append(qk_group(wsb_q, qT, jt, tt))
                b0_gens.append(qk_group(wsb_k, kT, jt, tt))
        for tt in range(8):
            for jh in range(2):
                b0_gens.append(v_group(wsb_v, tt, jh))
        run_all(b0_gens)

        # ---- feeder queues for the attention phases ----
        # b0-attention interleave: early-needed part of b1's q/k/v projections.
        feed_b0 = []
        for jt in (0, 1, 2, 3):
            for tt in (2, 3):
                feed_b0.append(qk_group(wsb_q, qT, jt, tt))
                feed_b0.append(qk_group(wsb_k, kT, jt, tt))
        for tt in range(8, 16):
            feed_b0.append(v_group(wsb_v, tt, 0))
        # b1-attention interleave: late q/k jt-groups + v jh=1 + o-proj(b0).
        feed_b1 = []
        for jt in (4, 5):
            for tt in (2, 3):
                feed_b1.append(qk_group(wsb_q, qT, jt, tt))
                feed_b1.append(qk_group(wsb_k, kT, jt, tt))
        for tt in range(8, 16):
            feed_b1.append(v_group(wsb_v, tt, 1))
        for jt in (6, 7):
            for tt in (2, 3):
                feed_b1.append(qk_group(wsb_q, qT, jt, tt))
                feed_b1.append(qk_group(wsb_k, kT, jt, tt))

        b0_steps = len(feed_b0) * 5
        b1_steps = len(feed_b1) * 5 + 8 * 11

        pa.release()

        # ================= attention =================
        pb = tc.alloc_tile_pool(name="pb", bufs=2)

        def stage0(b, hp, k, qk, kt, vv):
            """R matmuls + skew bounce write + skewed msk read issue.
            Runs 2 units ahead of stage1 so the DRAM round trip never
            stalls the PE queue."""
            t0 = 128 * k
            r0 = 897 - t0
            msks = []
            segs = []
            for h in (0, 1):
                hg = 2 * hp + h
                bh_base = (b * H + hg) * RSEG
                segs.append(((b * H + hg) * NBLK + k) * ASEG)
                tp = (64 * h, 0)
                lhs_q = qk[64 * h : 64 * h + 64, t0 : t0 + 128]

                # R[t, r] = q[t]·Tk[r] over the block window, bounced via DRAM
                rsb = pb.tile([128, WREL], F8, tag=f"rsb{h}")
                for ci, (c0, cw) in enumerate(((0, 512), (512, 512), (1024, 127))):
                    cm = 128 if cw == 127 else cw
                    ps = ps512.tile([128, 512], FP, tag="ps512")
                    nc.tensor.matmul(
                        ps[:, 0:cm],
                        lhsT=lhs_q,
                        rhs=tkT[64 * h : 64 * h + 64, r0 + c0 : r0 + c0 + cm],
                        start=True,
                        stop=True,
                        tile_position=tp,
                    )
                    nc.vector.tensor_copy(rsb[:, c0 : c0 + cw], ps[:, 0:cw])
                nc.gpsimd.dma_start(
                    out=_ap(rbuf, bh_base + t0 * 2048 + 897, [[2049, 128], [1, WREL]]),
                    in_=rsb,
                )

                # skewed read-back: msk[i, s] = R[t0+i, s - (t0+i) + 1024]
                msk = pb.tile([128, T], F8, tag=f"msk{h}", bufs=3)
                nc.gpsimd.dma_start(
                    out=msk,
                    in_=_ap(rbuf, bh_base + t0 * 2048 + 1024, [[2048, 128], [1, T]]),
                )
                msks.append(msk)
            return dict(b=b, hp=hp, k=k, qk=qk, kt=kt, vv=vv, msks=msks, segs=segs)

        def stage1(b, hp, k, qk, kt, vv, msks, segs, feed):
            """sim(+msk via PE identity-accum) + exp from PSUM (bf16) +
            abuf write + skewed askw read issue + 1/l broadcast row."""
            t0 = 128 * k
            ats = []
            askts = []
            recs = pb.tile([1, 256], FP, tag="recs")
            rb128 = pb.tile([128, 256], FP, tag="rb128")
            for h in (0, 1):
                tp = (64 * h, 0)
                lhs_q = qk[64 * h : 64 * h + 64, t0 : t0 + 128]
                lsums = []
                abf = pb.tile([128, T], BF, tag=f"abf{h}")
                for n in range(2):
                    ps = ps512.tile([128, 512], FP, tag="ps512")
                    nc.tensor.matmul(
                        ps,
                        lhsT=lhs_q,
                        rhs=kt[64 * h : 64 * h + 64, n * 512 : (n + 1) * 512],
                        start=True,
                        stop=False,
                        tile_position=tp,
                    )
                    nc.tensor.matmul(
                        ps,
                        lhsT=ident_f8,
                        rhs=msks[h][:, n * 512 : (n + 1) * 512],
                        start=False,
                        stop=True,
                        skip_group_check=True,
                    )
                    lsum_n = pb.tile([128, 1], FP, tag=f"lsum{h}_{n}")
                    nc.scalar.activation(
                        out=abf[:, n * 512 : (n + 1) * 512], in_=ps,
                        func=mybir.ActivationFunctionType.Exp,
                        scale=float(SCALE), accum_out=lsum_n,
                    )
                    lsums.append(lsum_n)
                lsum = pb.tile([128, 1], FP, tag=f"lsum{h}")
                nc.scalar.activation(
                    out=lsum, in_=lsums[0],
                    func=mybir.ActivationFunctionType.Identity,
                    bias=lsums[1], scale=1.0,
                )
                nc.scalar.dma_start(
                    out=_ap(abuf, segs[h] + AGUARD, [[1024, 128], [1, 1024]]), in_=abf
                )

                # e^T and skewed-e^T read back through the DMA xbar transpose
                # (DRAM source path; the corner-mask predicates are symmetric
                # under transpose, so they apply unchanged to askt).
                at = pb.tile([128, 8, 128], BF, tag=f"at{h}")
                nc.sync.dma_start_transpose(
                    out=at, in_=_ap(abuf, segs[h] + AGUARD, [[1024, 128], [1, 1024]])
                )
                askt = pb.tile([128, 9, 128], BF, tag=f"askt{h}")
                nc.sync.dma_start_transpose(
                    out=askt, in_=_ap(abuf, segs[h] + 1, [[1025, 128], [1, 1152]])
                )
                nc.vector.copy_predicated(askt[:, 0, :], mask_hi, zeros_bf)
                nc.vector.copy_predicated(askt[:, 8, :], mask_lo, zeros_bf)
                ats.append(at)
                askts.append(askt)

                # 1/l as a broadcast row into rb128[64h:64h+64, :]
                recl = pb.tile([128, 1], FP, tag=f"recl{h}")
                nc.vector.reciprocal(recl, lsum)
                pl = ps512.tile([128, 512], FP, tag="ps512")
                nc.tensor.transpose(pl[0:1, 0:128], recl, ident)
                nc.vector.tensor_copy(recs[0:1, 128 * h : 128 * h + 128], pl[0:1, 0:128])
                feed()
            nc.gpsimd.partition_broadcast(rb128, recs)
            return dict(b=b, hp=hp, k=k, vv=vv, ats=ats, askts=askts, rb128=rb128)

        def stage2(b, hp, k, vv, ats, askts, rb128, feed):
            """attn@v + rel_v from the xbar-transposed tiles, accumulate
            out^T in PSUM, normalize by 1/l on evict, write to aot (bf16)."""
            po = pso.tile([128, 128], FP, tag="po")
            for h in (0, 1):
                feed()
                for c in range(8):
                    nc.tensor.matmul(
                        po[64 * h : 64 * h + 64, :],
                        lhsT=vv[:, c, 64 * h : 64 * h + 64],
                        rhs=ats[h][:, c, :],
                        start=(c == 0),
                        stop=False,
                        tile_position=(0, 64 * h),
                    )
                feed()
                for c in range(9):
                    nc.tensor.matmul(
                        po[64 * h : 64 * h + 64, :],
                        lhsT=tv_bf[:, 7 - k + c, :],
                        rhs=askts[h][:, c, :],
                        start=False,
                        stop=(c == 8),
                        tile_position=(0, 64 * h),
                    )
            ot = pb.tile([128, 128], BF, tag="ot")
            nc.vector.tensor_mul(ot[0:64, :], po[0:64, :], rb128[0:64, 0:128])
            nc.vector.tensor_mul(ot[64:128, :], po[64:128, :], rb128[64:128, 128:256])
            nc.sync.dma_start(
                out=aot[128 * hp : 128 * (hp + 1), b * T + 128 * k : b * T + 128 * (k + 1)],
                in_=ot,
            )

        # ---- main attention loop with feeder interleave ----
        class Feeder:
            """Meters projection work into the attention instruction stream.

            Each step of a generator emits ~2 matmuls.  `rate` is steps per
            feed() call, chosen so the queue lasts the whole phase (HAM needs
            real-matmul activity in every ~3.4us window)."""

            def __init__(self):
                self.it = iter(())
                self.rate = 1.0
                self.credit = 0.0

            def set(self, gens, rate):
                self.drain()
                self.it = chain.from_iterable(gens)
                self.rate = rate
                self.credit = 0.0

            def append(self, gens):
                self.it = chain(self.it, chain.from_iterable(gens))

            def feed(self):
                self.credit += self.rate
                while self.credit >= 1.0:
                    self.credit -= 1.0
                    if next(self.it, None) is None:
                        return

            def drain(self):
                for _ in self.it:
                    pass

        feeder = Feeder()
        wsb_o = None

        # flat unit list; 3-deep pipeline: stage0(i) | stage1(i-2) | stage2(i-3)
        units = [
            (b, hp, k)
            for b in range(num_b)
            for hp in range(num_hp)
            for k in range(num_blk)
        ]
        nfeeds_per_unit = 6
        loads = {}

        def get_bhp(b, hp):
            if (b, hp) not in loads:
                qk = pb.tile([128, T], BF, tag="qk")
                nc.sync.dma_start(
                    out=qk, in_=qT[128 * hp : 128 * (hp + 1), b * T : (b + 1) * T]
                )
                kt = pb.tile([128, T], BF, tag="kt")
                nc.sync.dma_start(
                    out=kt, in_=kT[128 * hp : 128 * (hp + 1), b * T : (b + 1) * T]
                )
                vv = pb.tile([128, 8, 128], BF, tag="vv")
                nc.sync.dma_start(
                    out=vv,
                    in_=vB[b * T : (b + 1) * T, 128 * hp : 128 * (hp + 1)].rearrange(
                        "(c p) d -> p c d", p=128
                    ),
                )
                loads[(b, hp)] = (qk, kt, vv)
            return loads[(b, hp)]

        p0 = []  # stage0 done, awaiting stage1
        p1 = []  # stage1 done, awaiting stage2
        nb0 = sum(1 for u in units if u[0] == 0)

        def pump(i, u):
            nonlocal wsb_o
            if u is not None:
                b, hp, k = u
                if i == 0:
                    feeder.set(feed_b0, rate=b0_steps / (nb0 * nfeeds_per_unit))
                if i == nb0:
                    feeder.set(feed_b1, rate=b1_steps / (nb0 * nfeeds_per_unit))
                if b == 1 and hp == 4 and k == 0 and num_hp == NHP:
                    wsb_o = load_w(wo, "wsb_vo")
                    feeder.append([oproj_block(wsb_o, 0, kk) for kk in range(8)])
                qk, kt, vv = get_bhp(b, hp)
                feeder.feed()
                p0.append(stage0(b, hp, k, qk, kt, vv))
                feeder.feed()
                if k == 4 and i + 4 < len(units):
                    nb, nhp, _ = units[i + 4]
                    get_bhp(nb, nhp)
            if (u is None and p0) or len(p0) > 2:
                p1.append(stage1(**p0.pop(0), feed=feeder.feed))
            if (u is None and p1 and not p0) or len(p1) > 1:
                stage2(**p1.pop(0), feed=feeder.feed)

        for i, u in enumerate(units):
            pump(i, u)
        while p0 or p1:
            pump(None, None)
        feeder.drain()

        # tail: b1's output projection, dense
        if wsb_o is None:
            wsb_o = load_w(wo, "wsb_vo")
        run_all([oproj_block(wsb_o, 1, k) for k in range(min(num_blk, 8))])

        pb.release()
        pso.release()
        pev.release()
        pw.release()
        ps512.release()
        const.release()

    nc.compile()
    return nc


_NC_CACHE = None


def _make_in_maps(x, Wq, Wk, Wv, Wo, bo, rel_k_table, rel_v_table):
    f32 = lambda a: np.ascontiguousarray(np.asarray(a, dtype=np.float32))
    x = f32(x).reshape(NCORE, BL, T, D)
    shared = dict(
        Wq=f32(Wq), Wk=f32(Wk), Wv=f32(Wv), Wo=f32(Wo), bo=f32(bo),
        rel_k_table=f32(rel_k_table), rel_v_table=f32(rel_v_table),
    )
    return [dict(x=np.ascontiguousarray(x[i]), **shared) for i in range(NCORE)]


def kernel(x, Wq, Wk, Wv, Wo, bo, rel_k_table, rel_v_table):
    global _NC_CACHE
    if _NC_CACHE is None:
        _NC_CACHE = build()
    in_maps = _make_in_maps(x, Wq, Wk, Wv, Wo, bo, rel_k_table, rel_v_table)
    res = run_bass_kernel_spmd(_NC_CACHE, in_maps, list(range(NCORE)))
    out = np.concatenate([res.results[i]["y"] for i in range(NCORE)], axis=0)
    return out.reshape(B, T, D).astype(np.float32)


# revision 19
# speedup vs baseline: 1.3523x; 1.3523x over previous
"""Trainium2 Bass kernel for BasicTransformerBlock_Epipolar (relative-position attention).

Math (per batch b, head h):
  q = x@Wq, k = x@Wk, v = x@Wv            (H=16 heads, dh=64)
  sim[t,s]  = (q[t]·k[s] + q[t]·Tk[s-t+1024]) * dh^-0.5
  attn      = softmax_s(sim)
  out[t]    = sum_s attn[t,s]*v[s] + sum_s attn[t,s]*Tv[s-t+1024]
  y = out@Wo + bo

Sharding: data-parallel over batch, 2 batches per core, 8 cores, no collectives.

v2 notes (vs the first working version):
  - The PE's HAM clock gate ignores transpose-mode activity, so an attention
    phase whose real matmuls come in sparse bursts runs at K=4/8 (1.2 GHz).
    Projection matmul groups are interleaved into the attention stream
    ("feeder") so every HAM window sees real matmul activity: b0-attention
    carries part of b1's q/k/v projections, b1-attention carries the rest
    plus b0's output projection; b1's output projection forms a dense tail.
  - The rel-k skew addition (sim += msk) is done on the PE as an
    identity-matmul accumulation into the sim PSUM instead of a DVE
    tensor_tensor, and exp() reads the PSUM directly.
  - Bounce buffers: rbuf in fp8e4 (R values are O(1)), abuf in bf16.
    qT/kT/aot/weights in bf16.

The relative-position terms need a "skew" (diagonal remap) which SBUF access
patterns cannot express; both are routed through DRAM with strided access
patterns:
  - R[t,r] = q[t]·Tk[r] is computed blockwise as a plain matmul, written to a
    row-stride-2049 buffer (fp8), and read back with row stride 2048, which
    yields exactly M[t,s] = R[t, s-t+1024].
  - e = exp(SCALE*sim) (unnormalized attn) is written contiguously (row
    stride 1024, bf16) and read back with row stride 1025, which yields
    A_skew[i,j] = e[i, i+j-127]; PE-transposed chunks of it contract with Tv
    into the same PSUM as attn@v.  Softmax normalization is deferred to the
    PSUM eviction (one multiply by a broadcast 1/l row).
"""

import sys

sys.path.insert(0, "/opt/trn_rl_repo")

from itertools import chain

import numpy as np

import concourse.bass as bass
import concourse.tile as tile
from concourse import bacc, mybir
from concourse.bass_utils import run_bass_kernel_spmd
from concourse.masks import make_identity

FP = mybir.dt.float32
FR = mybir.dt.float32r
BF = mybir.dt.bfloat16
F8 = mybir.dt.float8e4

B, T, D = 16, 1024, 1024
H, DH = 16, 64
NCORE = 8
BL = B // NCORE          # batches per core
TL = BL * T              # local token rows
SCALE = DH ** -0.5
NHP = H // 2             # head pairs
NBLK = T // 128          # 128-row blocks per batch
WREL = 1151              # rel window width per 128 t-block
RSTRIDE = 2049
RSEG = T * RSTRIDE       # rbuf elements per (b, h)
AGUARD = 128
ASEG = AGUARD + 128 * 1024 + AGUARD  # abuf elements per (b, h, blk)


def _ap(t_ap, offset, pattern):
    return bass.AP(tensor=t_ap.tensor, offset=offset, ap=pattern)


def build(num_b=BL, num_hp=NHP, num_blk=NBLK):
    nc = bacc.Bacc("TRN2", target_bir_lowering=False, debug=False, num_devices=NCORE)

    x = nc.dram_tensor("x", [BL, T, D], FP, kind="ExternalInput").ap()
    wq = nc.dram_tensor("Wq", [D, D], FP, kind="ExternalInput").ap()
    wk = nc.dram_tensor("Wk", [D, D], FP, kind="ExternalInput").ap()
    wv = nc.dram_tensor("Wv", [D, D], FP, kind="ExternalInput").ap()
    wo = nc.dram_tensor("Wo", [D, D], FP, kind="ExternalInput").ap()
    bo = nc.dram_tensor("bo", [D], FP, kind="ExternalInput").ap()
    tk = nc.dram_tensor("rel_k_table", [2 * T + 1, DH], FP, kind="ExternalInput").ap()
    tv = nc.dram_tensor("rel_v_table", [2 * T + 1, DH], FP, kind="ExternalInput").ap()
    y = nc.dram_tensor("y", [BL, T, D], FP, kind="ExternalOutput").ap()

    qT = nc.dram_tensor("qT", [D, TL], BF).ap()
    kT = nc.dram_tensor("kT", [D, TL], BF).ap()
    vB = nc.dram_tensor("vB", [TL, D], BF).ap()
    aot = nc.dram_tensor("aot", [D, TL], BF).ap()
    rbuf = nc.dram_tensor("rbuf", [BL * H * RSEG], F8).ap()
    abuf = nc.dram_tensor("abuf", [BL * H * NBLK * ASEG], BF).ap()

    with tile.TileContext(nc) as tc:
        const = tc.alloc_tile_pool(name="const", bufs=1)
        ps512 = tc.alloc_tile_pool(name="ps512", bufs=4, space="PSUM")
        pstb = tc.alloc_tile_pool(name="pstb", bufs=3, space="PSUM")
        pso = tc.alloc_tile_pool(name="pso", bufs=1, space="PSUM")

        # ---- constants ----
        ident = const.tile([128, 128], FP, tag="ident")
        make_identity(nc, ident)
        ident_bf = const.tile([128, 128], BF, tag="ident_bf")
        nc.vector.tensor_copy(ident_bf, ident)
        ident_f8 = const.tile([128, 128], F8, tag="ident_f8")
        nc.vector.tensor_copy(ident_f8, ident)

        ones_i8 = const.tile([128, 128], mybir.dt.int8, tag="ones_i8")
        nc.vector.memset(ones_i8, 1)
        zeros_bf = const.tile([128, 128], BF, tag="zeros_bf")
        nc.vector.memset(zeros_bf, 0.0)
        # mask_lo[p,f] = 1 if p+f >= 127 ; mask_hi[p,f] = 1 if p+f <= 126
        # invalid corners of the skewed-attn read: chunk 0 is invalid where
        # p+f <= 126 (use mask_hi to zero), chunk 8 invalid where p+f >= 127.
        mask_lo = const.tile([128, 128], mybir.dt.int8, tag="mask_lo")
        nc.gpsimd.affine_select(
            out=mask_lo, in_=ones_i8, pattern=[[1, 128]],
            compare_op=mybir.AluOpType.is_ge, fill=0, base=-127,
            channel_multiplier=1,
        )
        mask_hi = const.tile([128, 128], mybir.dt.int8, tag="mask_hi")
        nc.gpsimd.affine_select(
            out=mask_hi, in_=ones_i8, pattern=[[-1, 128]],
            compare_op=mybir.AluOpType.is_ge, fill=0, base=126,
            channel_multiplier=-1,
        )

        # bo broadcast to all partitions
        bo128 = const.tile([128, D], FP, tag="bo128")
        nc.sync.dma_start(out=bo128, in_=_ap(bo, 0, [[0, 128], [1, D]]))

        # ================= weights + x^T residency =================
        pw = tc.alloc_tile_pool(name="pw", bufs=1)
        pev = tc.alloc_tile_pool(name="pev", bufs=4)
        pa = tc.alloc_tile_pool(name="pa", bufs=2)

        # Tk^T resident in SBUF (bf16), duplicated across both partition
        # halves so it can feed row-packed matmuls for either head.
        tk_tmp = pa.tile([128, 16, DH], FP, tag="tk_tmp")
        nc.sync.dma_start(
            out=tk_tmp, in_=tk[0 : 16 * 128, :].rearrange("(c p) d -> p c d", p=128)
        )
        tkT = const.tile([128, 16 * 128 + 4], BF, tag="tkT")
        for c in range(16):
            ptile = ps512.tile([128, 512], FP, tag="ps512")
            nc.tensor.transpose(ptile[0:DH, 0:128], tk_tmp[:, c, :], ident)
            nc.scalar.copy(out=tkT[0:DH, c * 128 : (c + 1) * 128], in_=ptile[0:DH, 0:128])
            nc.scalar.copy(out=tkT[DH:128, c * 128 : (c + 1) * 128], in_=ptile[0:DH, 0:128])

        nc.scalar.copy(out=tkT[:, 16 * 128 : 16 * 128 + 4], in_=zeros_bf[:, 0:4])

        # Tv rows 1..2048 as bf16 chunks: tv_bf[p, m, d] = tv[1 + 128m + p, d]
        tv_tmp = pa.tile([128, 16, DH], FP, tag="tv_tmp")
        nc.sync.dma_start(
            out=tv_tmp, in_=tv[1 : 1 + 16 * 128, :].rearrange("(m p) d -> p m d", p=128)
        )
        tv_bf = const.tile([128, 16, DH], BF, tag="tv_bf")
        nc.scalar.copy(out=tv_bf, in_=tv_tmp)

        # x^T resident (bf16): xT[p, ic, tg] = x[tg//T, tg%T, 128*ic + p]
        xT = pw.tile([128, 8, TL], BF, tag="xT")
        for b in range(num_b):
            for tb in range(8):
                xt = pa.tile([128, D], FP, tag="xt")
                nc.sync.dma_start(out=xt, in_=x[b, tb * 128 : (tb + 1) * 128, :])
                for ic in range(8):
                    ptile = ps512.tile([128, 512], FP, tag="ps512")
                    nc.tensor.transpose(ptile[:, 0:128], xt[:, ic * 128 : (ic + 1) * 128], ident)
                    nc.scalar.copy(
                        out=xT[:, ic, (b * 8 + tb) * 128 : (b * 8 + tb + 1) * 128],
                        in_=ptile[:, 0:128],
                    )

        # weights resident in bf16, cast f32->bf16 in the DMA engine (SWDGE);
        # the wv slot is later reused for wo.
        def load_w(w_ap, tag):
            wsb = pw.tile([128, 8, D], BF, tag=tag)
            nc.gpsimd.dma_start(out=wsb, in_=w_ap.rearrange("(c p) j -> p c j", p=128))
            return wsb

        wsb_q = load_w(wq, "wsb_q")
        wsb_k = load_w(wk, "wsb_k")
        wsb_v = load_w(wv, "wsb_vo")

        # ---- projection group generators (each yield ≈ 2 matmuls) ----
        def qk_group(wsb, dst, jt, tt):
            ps = ps512.tile([128, 512], FP, tag="ps512")
            for ic2 in range(4):
                for ic in (2 * ic2, 2 * ic2 + 1):
                    nc.tensor.matmul(
                        ps,
                        lhsT=wsb[:, ic, jt * 128 : (jt + 1) * 128],
                        rhs=xT[:, ic, tt * 512 : (tt + 1) * 512],
                        start=(ic == 0),
                        stop=(ic == 7),
                    )
                yield
            ev = pev.tile([128, 512], BF, tag="ev")
            nc.scalar.copy(out=ev, in_=ps)
            nc.sync.dma_start(
                out=dst[jt * 128 : (jt + 1) * 128, tt * 512 : (tt + 1) * 512],
                in_=ev,
            )
            yield

        def v_group(wsb, tt, jh):
            ps = ps512.tile([128, 512], FP, tag="ps512")
            for ic2 in range(4):
                for ic in (2 * ic2, 2 * ic2 + 1):
                    nc.tensor.matmul(
                        ps,
                        lhsT=xT[:, ic, tt * 128 : (tt + 1) * 128],
                        rhs=wsb[:, ic, jh * 512 : (jh + 1) * 512],
                        start=(ic == 0),
                        stop=(ic == 7),
                    )
                yield
            ev = pev.tile([128, 512], BF, tag="evb")
            nc.scalar.copy(out=ev, in_=ps)
            nc.sync.dma_start(
                out=vB[tt * 128 : (tt + 1) * 128, jh * 512 : (jh + 1) * 512],
                in_=ev,
            )
            yield

        def oproj_block(wsb, b, k):
            tt = b * 8 + k
            asb = pev.tile([128, 8, 128], BF, tag="asb")
            nc.sync.dma_start(
                out=asb,
                in_=aot[:, tt * 128 : (tt + 1) * 128].rearrange(
                    "(c p) t -> p c t", p=128
                ),
            )
            yield
            for eh in range(2):
                ps = ps512.tile([128, 512], FP, tag="ps512")
                for ic2 in range(4):
                    for ic in (2 * ic2, 2 * ic2 + 1):
                        nc.tensor.matmul(
                            ps,
                            lhsT=asb[:, ic, :],
                            rhs=wsb[:, ic, eh * 512 : (eh + 1) * 512],
                            start=(ic == 0),
                            stop=(ic == 7),
                        )
                    yield
                ysb = pev.tile([128, 512], FP, tag="ysb")
                nc.vector.tensor_add(ysb, ps, bo128[:, eh * 512 : (eh + 1) * 512])
                nc.sync.dma_start(
                    out=y[b, k * 128 : (k + 1) * 128, eh * 512 : (eh + 1) * 512],
                    in_=ysb,
                )
                yield

        # ---- phase A: b0's projections, emitted densely ----
        def run_all(gens):
            for g in gens:
                for _ in g:
                    pass

        b0_gens = []
        for jt in range(8):
            for tt in (0, 1):
                b0_gens.append(qk_group(wsb_q, qT, jt, tt))
                b0_gens.append(qk_group(wsb_k, kT, jt, tt))
        for tt in range(8):
            for jh in range(2):
                b0_gens.append(v_group(wsb_v, tt, jh))
        run_all(b0_gens)

        # ---- feeder queues for the attention phases ----
        # b0-attention interleave: early-needed part of b1's q/k/v projections.
        feed_b0 = []
        for jt in (0, 1, 2, 3):
            for tt in (2, 3):
                feed_b0.append(qk_group(wsb_q, qT, jt, tt))
                feed_b0.append(qk_group(wsb_k, kT, jt, tt))
        for tt in range(8, 16):
            feed_b0.append(v_group(wsb_v, tt, 0))
        # b1-attention interleave: late q/k jt-groups + v jh=1 + o-proj(b0).
        feed_b1 = []
        for jt in (4, 5):
            for tt in (2, 3):
                feed_b1.append(qk_group(wsb_q, qT, jt, tt))
                feed_b1.append(qk_group(wsb_k, kT, jt, tt))
        for tt in range(8, 16):
            feed_b1.append(v_group(wsb_v, tt, 1))
        for jt in (6, 7):
            for tt in (2, 3):
                feed_b1.append(qk_group(wsb_q, qT, jt, tt))
                feed_b1.append(qk_group(wsb_k, kT, jt, tt))

        b0_steps = len(feed_b0) * 5
        b1_steps = len(feed_b1) * 5 + 8 * 11

        pa.release()

        # ================= attention =================
        pb = tc.alloc_tile_pool(name="pb", bufs=2)

        def stage0(b, hp, k, qk, kt, vv):
            """R matmuls + skew bounce write + skewed msk read issue.
            Runs 2 units ahead of stage1 so the DRAM round trip never
            stalls the PE queue."""
            t0 = 128 * k
            r0 = 897 - t0
            msks = []
            segs = []
            for h in (0, 1):
                hg = 2 * hp + h
                bh_base = (b * H + hg) * RSEG
                segs.append(((b * H + hg) * NBLK + k) * ASEG)
                tp = (64 * h, 0)
                lhs_q = qk[64 * h : 64 * h + 64, t0 : t0 + 128]

                # R[t, r] = q[t]·Tk[r] over the block window, bounced via DRAM
                rsb = pb.tile([128, WREL], F8, tag=f"rsb{h}")
                for ci, (c0, cw) in enumerate(((0, 512), (512, 512), (1024, 127))):
                    cm = 128 if cw == 127 else cw
                    ps = ps512.tile([128, 512], FP, tag="ps512")
                    nc.tensor.matmul(
                        ps[:, 0:cm],
                        lhsT=lhs_q,
                        rhs=tkT[64 * h : 64 * h + 64, r0 + c0 : r0 + c0 + cm],
                        start=True,
                        stop=True,
                        tile_position=tp,
                    )
                    if ci < 2:
                        nc.scalar.copy(out=rsb[:, c0 : c0 + cw], in_=ps[:, 0:cw])
                    else:
                        nc.vector.tensor_copy(rsb[:, c0 : c0 + cw], ps[:, 0:cw])
                nc.gpsimd.dma_start(
                    out=_ap(rbuf, bh_base + t0 * 2048 + 897, [[2049, 128], [1, WREL]]),
                    in_=rsb,
                )

                # skewed read-back: msk[i, s] = R[t0+i, s - (t0+i) + 1024]
                msk = pb.tile([128, T], F8, tag=f"msk{h}", bufs=4)
                nc.gpsimd.dma_start(
                    out=msk,
                    in_=_ap(rbuf, bh_base + t0 * 2048 + 1024, [[2048, 128], [1, T]]),
                )
                msks.append(msk)
            return dict(b=b, hp=hp, k=k, qk=qk, kt=kt, vv=vv, msks=msks, segs=segs)

        def stage1(b, hp, k, qk, kt, vv, msks, segs, feed):
            """sim(+msk via PE identity-accum) + exp from PSUM (bf16) +
            abuf write + skewed askw read issue + 1/l broadcast row."""
            t0 = 128 * k
            abfs = []
            askws = []
            recs = pb.tile([1, 256], FP, tag="recs")
            rb128 = pb.tile([128, 256], FP, tag="rb128")
            for h in (0, 1):
                tp = (64 * h, 0)
                lhs_q = qk[64 * h : 64 * h + 64, t0 : t0 + 128]
                lsums = []
                abf = pb.tile([128, T], BF, tag=f"abf{h}")
                for n in range(2):
                    ps = ps512.tile([128, 512], FP, tag="ps512")
                    nc.tensor.matmul(
                        ps,
                        lhsT=lhs_q,
                        rhs=kt[64 * h : 64 * h + 64, n * 512 : (n + 1) * 512],
                        start=True,
                        stop=True,
                        tile_position=tp,
                    )
                    nc.vector.tensor_add(ps, ps, msks[h][:, n * 512 : (n + 1) * 512])
                    lsum_n = pb.tile([128, 1], FP, tag=f"lsum{h}_{n}")
                    nc.scalar.activation(
                        out=abf[:, n * 512 : (n + 1) * 512], in_=ps,
                        func=mybir.ActivationFunctionType.Exp,
                        scale=float(SCALE), accum_out=lsum_n,
                    )
                    lsums.append(lsum_n)
                lsum = pb.tile([128, 1], FP, tag=f"lsum{h}")
                nc.scalar.activation(
                    out=lsum, in_=lsums[0],
                    func=mybir.ActivationFunctionType.Identity,
                    bias=lsums[1], scale=1.0,
                )
                nc.scalar.dma_start(
                    out=_ap(abuf, segs[h] + AGUARD, [[1024, 128], [1, 1024]]), in_=abf
                )

                # issue the skewed read now; consumed by stage2 next unit
                askw = pb.tile([128, 1152], BF, tag=f"askw{h}")
                nc.sync.dma_start(
                    out=askw, in_=_ap(abuf, segs[h] + 1, [[1025, 128], [1, 1152]])
                )
                nc.vector.copy_predicated(askw[:, 0:128], mask_hi, zeros_bf)
                nc.vector.copy_predicated(askw[:, 1024:1152], mask_lo, zeros_bf)
                abfs.append(abf)
                askws.append(askw)

                # 1/l as a broadcast row into rb128[64h:64h+64, :]
                recl = pb.tile([128, 1], FP, tag=f"recl{h}")
                nc.vector.reciprocal(recl, lsum)
                pl = ps512.tile([128, 512], FP, tag="ps512")
                nc.tensor.transpose(pl[0:1, 0:128], recl, ident)
                nc.vector.tensor_copy(recs[0:1, 128 * h : 128 * h + 128], pl[0:1, 0:128])
                feed()
            nc.gpsimd.partition_broadcast(rb128, recs)
            return dict(b=b, hp=hp, k=k, vv=vv, abfs=abfs, askws=askws, rb128=rb128)

        def stage2(b, hp, k, vv, abfs, askws, rb128, feed):
            """e^T via PE transpose + attn@v + rel_v, accumulate out^T in
            PSUM, normalize by 1/l on evict, write to aot (bf16)."""
            po = pso.tile([128, 128], FP, tag="po")
            for h in (0, 1):
                at = pb.tile([128, 8, 128], BF, tag=f"at{h}")
                ptile = pstb.tile([128, 8, 128], BF, tag="pstb")
                for c8 in range(8):
                    nc.tensor.transpose(
                        ptile[:, c8, :],
                        abfs[h][:, c8 * 128 : (c8 + 1) * 128],
                        ident_bf,
                    )
                    if c8 == 3:
                        feed()
                nc.vector.tensor_copy(at, ptile)
                feed()
                for c in range(8):
                    nc.tensor.matmul(
                        po[64 * h : 64 * h + 64, :],
                        lhsT=vv[:, c, 64 * h : 64 * h + 64],
                        rhs=at[:, c, :],
                        start=(c == 0),
                        stop=False,
                        tile_position=(0, 64 * h),
                    )
                askt = pb.tile([128, 9, 128], BF, tag=f"askt{h}")
                ptile2 = pstb.tile([128, 8, 128], BF, tag="pstb")
                for c8 in range(8):
                    nc.tensor.transpose(
                        ptile2[:, c8, :],
                        askws[h][:, c8 * 128 : (c8 + 1) * 128],
                        ident_bf,
                    )
                    if c8 == 3:
                        feed()
                nc.vector.tensor_copy(askt[:, 0:8, :], ptile2)
                ptile3 = pstb.tile([128, 8, 128], BF, tag="pstb")
                nc.tensor.transpose(ptile3[:, 0, :], askws[h][:, 1024:1152], ident_bf)
                nc.vector.tensor_copy(askt[:, 8, :], ptile3[:, 0, :])
                feed()
                for c in range(9):
                    nc.tensor.matmul(
                        po[64 * h : 64 * h + 64, :],
                        lhsT=tv_bf[:, 7 - k + c, :],
                        rhs=askt[:, c, :],
                        start=False,
                        stop=(c == 8),
                        tile_position=(0, 64 * h),
                    )
            ot = pb.tile([128, 128], BF, tag="ot")
            nc.vector.tensor_mul(ot[0:64, :], po[0:64, :], rb128[0:64, 0:128])
            nc.vector.tensor_mul(ot[64:128, :], po[64:128, :], rb128[64:128, 128:256])
            nc.sync.dma_start(
                out=aot[128 * hp : 128 * (hp + 1), b * T + 128 * k : b * T + 128 * (k + 1)],
                in_=ot,
            )

        # ---- main attention loop with feeder interleave ----
        class Feeder:
            """Meters projection work into the attention instruction stream.

            Each step of a generator emits ~2 matmuls.  `rate` is steps per
            feed() call, chosen so the queue lasts the whole phase (HAM needs
            real-matmul activity in every ~3.4us window)."""

            def __init__(self):
                self.it = iter(())
                self.rate = 1.0
                self.credit = 0.0

            def set(self, gens, rate):
                self.drain()
                self.it = chain.from_iterable(gens)
                self.rate = rate
                self.credit = 0.0

            def append(self, gens):
                self.it = chain(self.it, chain.from_iterable(gens))

            def feed(self):
                self.credit += self.rate
                while self.credit >= 1.0:
                    self.credit -= 1.0
                    if next(self.it, None) is None:
                        return

            def drain(self):
                for _ in self.it:
                    pass

        feeder = Feeder()
        wsb_o = None

        # flat unit list; 3-deep pipeline: stage0(i) | stage1(i-2) | stage2(i-3)
        units = [
            (b, hp, k)
            for b in range(num_b)
            for hp in range(num_hp)
            for k in range(num_blk)
        ]
        nfeeds_per_unit = 10
        loads = {}

        def get_bhp(b, hp):
            if (b, hp) not in loads:
                qk = pb.tile([128, T], BF, tag="qk")
                nc.sync.dma_start(
                    out=qk, in_=qT[128 * hp : 128 * (hp + 1), b * T : (b + 1) * T]
                )
                kt = pb.tile([128, T], BF, tag="kt")
                nc.sync.dma_start(
                    out=kt, in_=kT[128 * hp : 128 * (hp + 1), b * T : (b + 1) * T]
                )
                vv = pb.tile([128, 8, 128], BF, tag="vv")
                nc.sync.dma_start(
                    out=vv,
                    in_=vB[b * T : (b + 1) * T, 128 * hp : 128 * (hp + 1)].rearrange(
                        "(c p) d -> p c d", p=128
                    ),
                )
                loads[(b, hp)] = (qk, kt, vv)
            return loads[(b, hp)]

        p0 = []  # stage0 done, awaiting stage1
        p1 = []  # stage1 done, awaiting stage2
        nb0 = sum(1 for u in units if u[0] == 0)

        def pump(i, u):
            nonlocal wsb_o
            if u is not None:
                b, hp, k = u
                if i == 0:
                    feeder.set(feed_b0, rate=b0_steps / (nb0 * nfeeds_per_unit))
                if i == nb0:
                    feeder.set(feed_b1, rate=b1_steps / (nb0 * nfeeds_per_unit))
                if b == 1 and hp == 4 and k == 0 and num_hp == NHP:
                    wsb_o = load_w(wo, "wsb_vo")
                    feeder.append([oproj_block(wsb_o, 0, kk) for kk in range(8)])
                qk, kt, vv = get_bhp(b, hp)
                feeder.feed()
                p0.append(stage0(b, hp, k, qk, kt, vv))
                feeder.feed()
                if k == 4 and i + 4 < len(units):
                    nb, nhp, _ = units[i + 4]
                    get_bhp(nb, nhp)
            if (u is None and p0) or len(p0) > 3:
                p1.append(stage1(**p0.pop(0), feed=feeder.feed))
            if (u is None and p1 and not p0) or len(p1) > 1:
                stage2(**p1.pop(0), feed=feeder.feed)

        for i, u in enumerate(units):
            pump(i, u)
        while p0 or p1:
            pump(None, None)
        feeder.drain()

        # tail: b1's output projection, dense
        if wsb_o is None:
            wsb_o = load_w(wo, "wsb_vo")
        run_all([oproj_block(wsb_o, 1, k) for k in range(min(num_blk, 8))])

        pb.release()
        pso.release()
        pstb.release()
        pev.release()
        pw.release()
        ps512.release()
        const.release()

    nc.compile()
    return nc


_NC_CACHE = None


def _make_in_maps(x, Wq, Wk, Wv, Wo, bo, rel_k_table, rel_v_table):
    f32 = lambda a: np.ascontiguousarray(np.asarray(a, dtype=np.float32))
    x = f32(x).reshape(NCORE, BL, T, D)
    shared = dict(
        Wq=f32(Wq), Wk=f32(Wk), Wv=f32(Wv), Wo=f32(Wo), bo=f32(bo),
        rel_k_table=f32(rel_k_table), rel_v_table=f32(rel_v_table),
    )
    return [dict(x=np.ascontiguousarray(x[i]), **shared) for i in range(NCORE)]


def kernel(x, Wq, Wk, Wv, Wo, bo, rel_k_table, rel_v_table):
    global _NC_CACHE
    if _NC_CACHE is None:
        _NC_CACHE = build()
    in_maps = _make_in_maps(x, Wq, Wk, Wv, Wo, bo, rel_k_table, rel_v_table)
    res = run_bass_kernel_spmd(_NC_CACHE, in_maps, list(range(NCORE)))
    out = np.concatenate([res.results[i]["y"] for i in range(NCORE)], axis=0)
    return out.reshape(B, T, D).astype(np.float32)
